# revision 9
# baseline (speedup 1.0000x reference)
"""Despawn2D (8-level db-style DWT analysis + synthesis) on 8 Trainium2 cores.

Math: the reference's FFT circular convolutions with 4-tap filters reduce to
4-tap circular stencils (L = 8192 is a power of two, so the ReplicationPad is
a no-op).  Per level:

  analysis:  out[j]  = f0*a[2j] + f1*a[2j-1] + f2*a[2j-2] + f3*a[2j-3] (mod N)
             with f = h (approx) and f = g (detail), g = flip(h)*(+,-,+,-)
  synthesis: out[2i]   = g0*d[i]   + g2*d[i+1] + h0*r[i]   + h2*r[i+1]
             out[2i+1] = g1*d[i+1] + g3*d[i+2] + h1*r[i+1] + h3*r[i+2] (mod m)

Each tap is one fused multiply-accumulate instruction (scalar_tensor_tensor)
over a [128, M] tile; circular wrap is handled with small halo regions.

Sharding: pure data parallel — 2048 rows / 8 cores = 256 rows/core,
processed as 2 tiles of 128 partitions x 8192.

When the provided filter bank is orthogonal (it is for the db2 filters the
reference uses), synthesis(analysis(x)) == x exactly, so the "rec" output is
produced by a DMA copy of the input tile and only the analysis runs on the
compute engines.  A host-side fp64 check of the perfect-reconstruction
property on a small probe vector selects that fast path; otherwise a full
on-device synthesis variant is used.
"""

import numpy as np

LEVELS = 8
L = 8192
ROWS_TOTAL = 2048
N_CORES = 8
RPC = ROWS_TOTAL // N_CORES  # rows per core
P = 128  # SBUF partitions
NT = RPC // P  # tiles per core

_nc_cache = {}


def _make_g(h):
    g = h[::-1].copy()
    g[1::2] *= -1.0
    return g


def _taps_array(scaling):
    """(LEVELS*8,) row: per level [h0..h3, g0..g3], tiled to (P, LEVELS*8)."""
    row = np.empty(LEVELS * 8, np.float32)
    for lev in range(LEVELS):
        h = scaling[lev].astype(np.float32)
        g = _make_g(h)
        row[lev * 8: lev * 8 + 4] = h
        row[lev * 8 + 4: lev * 8 + 8] = g
    return np.tile(row, (P, 1)).copy()


def _pr_is_identity(scaling):
    """fp64 host check: does synthesis(analysis(x)) == x for these filters?"""
    rng = np.random.default_rng(1234)
    n0 = 1 << (LEVELS + 2)
    x = rng.standard_normal((2, n0))
    a = x.copy()
    details = []
    for lev in range(LEVELS):
        h = scaling[lev].astype(np.float64)
        g = _make_g(h)
        N = a.shape[1]
        idx = (np.arange(N // 2)[:, None] * 2 - np.arange(4)[None, :]) % N
        d = (a[:, idx] * g).sum(-1)
        a = (a[:, idx] * h).sum(-1)
        details.append(d)
    r = a
    for lev in reversed(range(LEVELS)):
        h = scaling[lev].astype(np.float64)
        g = _make_g(h)
        d = details[lev]
        m = r.shape[1]
        out = np.empty((2, 2 * m))
        i = np.arange(m)
        out[:, 0::2] = (g[0] * d[:, i] + g[2] * d[:, (i + 1) % m]
                        + h[0] * r[:, i] + h[2] * r[:, (i + 1) % m])
        out[:, 1::2] = (g[1] * d[:, (i + 1) % m] + g[3] * d[:, (i + 2) % m]
                        + h[1] * r[:, (i + 1) % m] + h[3] * r[:, (i + 2) % m])
        r = out
    err = np.abs(r - x).max() / max(np.abs(x).max(), 1e-30)
    return err < 1e-10


def _build(synth: bool):
    import concourse.bacc as bacc
    import concourse.mybir as mybir
    from concourse.tile import TileContext

    f32 = mybir.dt.float32
    Alu = mybir.AluOpType

    nc = bacc.Bacc()
    x = nc.dram_tensor("x", [RPC, L], f32, kind="ExternalInput")
    taps = nc.dram_tensor("taps", [P, LEVELS * 8], f32, kind="ExternalInput")
    rec = nc.dram_tensor("rec", [RPC, L], f32, kind="ExternalOutput")
    coeffs = nc.dram_tensor("coeffs", [RPC, L], f32, kind="ExternalOutput")

    # detail block offsets inside a coeffs row: [d0 | d1 | ... | d7 | a8]
    doff = []
    off = 0
    for lev in range(LEVELS):
        doff.append(off)
        off += L >> (lev + 1)
    aoff = off  # 8160

    with TileContext(nc) as tc:
        import contextlib
        with contextlib.ExitStack() as ctx:
            cpool = ctx.enter_context(tc.tile_pool(name="consts", bufs=1))
            xpool = ctx.enter_context(
                tc.tile_pool(name="xio", bufs=1 if synth else 2))
            wpool = ctx.enter_context(tc.tile_pool(name="work", bufs=1))
            spool = ctx.enter_context(tc.tile_pool(name="scratch", bufs=2))

            tp0 = cpool.tile([P, LEVELS * 8], f32)
            nc.sync.dma_start(out=tp0[:, :], in_=taps[:, :])
            # DVE-owned copy: ScalarE (ACT) instructions support only ONE
            # semaphore wait, so everything ACT touches must trace to DVE.
            tp = cpool.tile([P, LEVELS * 8], f32)
            nc.vector.tensor_copy(out=tp[:, :], in_=tp0[:, :])

            def tap(lev, k):  # h taps
                c = lev * 8 + k
                return tp[:, c:c + 1]

            def gtap(lev, k):  # g taps
                c = lev * 8 + 4 + k
                return tp[:, c:c + 1]

            for t in range(NT):
                rows = slice(t * P, (t + 1) * P)

                # input tile with 3-element left halo (circular wrap)
                xt = xpool.tile([P, L + 3], f32, tag="xt")
                nc.sync.dma_start(out=xt[:, 3:3 + L], in_=x[rows, :])
                nc.vector.tensor_copy(out=xt[:, 0:3], in_=xt[:, L:L + 3])

                if not synth:
                    # orthogonal filter bank: synthesis(analysis(x)) == x
                    nc.sync.dma_start(out=rec[rows, :], in_=xt[:, 3:3 + L])

                # ---------------- analysis ----------------
                a_ext = xt  # ext[i] == a[i-3]
                d_tiles = []
                a_last = None
                for lev in range(LEVELS):
                    N = L >> lev
                    M = N >> 1
                    last = lev == LEVELS - 1
                    if not last:
                        a_t = wpool.tile([P, M + 3], f32, tag=f"a{lev}")
                        a_main = a_t[:, 3:3 + M]
                    else:
                        a_t = wpool.tile([P, M + 2], f32, tag=f"a{lev}")
                        a_main = a_t[:, 0:M]
                    d_t = wpool.tile([P, M + 2], f32, tag=f"d{lev}")
                    d_main = d_t[:, 0:M]

                    for out_m, taps4, sl in (
                        (a_main, [tap(lev, k) for k in range(4)], "sa"),
                        (d_main, [gtap(lev, k) for k in range(4)], "sd"),
                    ):
                        # tap k reads ext[(3-k) + 2j], j = 0..M-1
                        # first tap -> scratch on ACT (except level 0, whose
                        # input xt is DMA-written: ACT only allows one wait);
                        # middle taps accumulate scratch on DVE; the last tap
                        # writes the real buffer (WAR vs its DMA reader then
                        # lands on DVE, which allows several waits).
                        s = spool.tile([P, M], f32, tag=sl)
                        if lev == 0:
                            nc.vector.tensor_scalar_mul(
                                s[:, 0:M], a_ext[:, 3:3 + N:2], taps4[0])
                        else:
                            nc.scalar.mul(
                                s[:, 0:M], a_ext[:, 3:3 + N:2], taps4[0])
                        for k in (1, 2):
                            nc.vector.scalar_tensor_tensor(
                                out=s[:, 0:M],
                                in0=a_ext[:, 3 - k:3 - k + N:2],
                                scalar=taps4[k],
                                in1=s[:, 0:M],
                                op0=Alu.mult,
                                op1=Alu.add,
                            )
                        nc.vector.scalar_tensor_tensor(
                            out=out_m,
                            in0=a_ext[:, 0:N:2],
                            scalar=taps4[3],
                            in1=s[:, 0:M],
                            op0=Alu.mult,
                            op1=Alu.add,
                        )

                    # details go straight out to HBM
                    nc.sync.dma_start(
                        out=coeffs[rows, doff[lev]:doff[lev] + M], in_=d_main)
                    if last:
                        nc.sync.dma_start(
                            out=coeffs[rows, aoff:aoff + M], in_=a_main)

                    if not last:
                        # left halo: ext[0:3] = a[M-3:M]
                        nc.vector.tensor_copy(
                            out=a_t[:, 0:3], in_=a_t[:, M:M + 3])
                    elif synth:
                        # right halo for synthesis start
                        nc.vector.tensor_copy(
                            out=a_t[:, M:M + 2], in_=a_t[:, 0:2])
                    if synth and not last:
                        pass  # d halos filled below only when needed
                    d_tiles.append(d_t)
                    a_ext = a_t
                    if last:
                        a_last = a_t

                # ---------------- synthesis ----------------
                if synth:
                    r_ext = a_last  # [P, m+2] with right halo
                    for lev in reversed(range(LEVELS)):
                        m = L >> (lev + 1)
                        d_t = d_tiles[lev]
                        # fill d right halo: d[m:m+2] = d[0:2]
                        nc.vector.tensor_copy(
                            out=d_t[:, m:m + 2], in_=d_t[:, 0:2])
                        if lev > 0:
                            o_t = wpool.tile([P, 2 * m + 2], f32, tag=f"r{lev}")
                            ev = o_t[:, 0:2 * m:2]
                            od = o_t[:, 1:2 * m:2]
                        else:
                            # final level: write into xt, then DMA to rec
                            o_t = xt
                            ev = xt[:, 3:3 + L:2]
                            od = xt[:, 4:4 + L - 1:2]
                        h4 = [tap(lev, k) for k in range(4)]
                        g4 = [gtap(lev, k) for k in range(4)]
                        # stride-2 output APs fault the ACT engine on HW, so
                        # the first tap runs on DVE here (outputs interleave)
                        first_mul = nc.vector.tensor_scalar_mul
                        # even: g0*d[i] + g2*d[i+1] + h0*r[i] + h2*r[i+1]
                        first_mul(ev, d_t[:, 0:m], g4[0])
                        for src, s in ((d_t[:, 1:m + 1], g4[2]),
                                       (r_ext[:, 0:m], h4[0]),
                                       (r_ext[:, 1:m + 1], h4[2])):
                            nc.vector.scalar_tensor_tensor(
                                out=ev, in0=src, scalar=s, in1=ev,
                                op0=Alu.mult, op1=Alu.add)
                        # odd: g1*d[i+1] + g3*d[i+2] + h1*r[i+1] + h3*r[i+2]
                        first_mul(od, d_t[:, 1:m + 1], g4[1])
                        for src, s in ((d_t[:, 2:m + 2], g4[3]),
                                       (r_ext[:, 1:m + 1], h4[1]),
                                       (r_ext[:, 2:m + 2], h4[3])):
                            nc.vector.scalar_tensor_tensor(
                                out=od, in0=src, scalar=s, in1=od,
                                op0=Alu.mult, op1=Alu.add)
                        if lev > 0:
                            nc.vector.tensor_copy(
                                out=o_t[:, 2 * m:2 * m + 2], in_=o_t[:, 0:2])
                            r_ext = o_t
                    nc.sync.dma_start(out=rec[rows, :], in_=xt[:, 3:3 + L])

    nc.finalize()
    return nc


def _get_nc(synth: bool):
    key = ("synth", synth)
    if key not in _nc_cache:
        _nc_cache[key] = _build(synth)
    return _nc_cache[key]


def kernel(x: np.ndarray, scaling: np.ndarray):
    from concourse.bass_utils import run_bass_kernel_spmd

    x = np.ascontiguousarray(np.asarray(x, np.float32))
    scaling = np.asarray(scaling, np.float32)
    assert x.shape == (ROWS_TOTAL, L), x.shape
    assert scaling.shape == (LEVELS, 4), scaling.shape

    synth = not _pr_is_identity(scaling)
    nc = _get_nc(synth)

    taps = _taps_array(scaling)
    in_maps = [
        {"x": np.ascontiguousarray(x[i * RPC:(i + 1) * RPC]), "taps": taps}
        for i in range(N_CORES)
    ]
    res = run_bass_kernel_spmd(nc, in_maps, core_ids=list(range(N_CORES)))
    outs = res.results
    rec = np.concatenate([outs[i]["rec"] for i in range(N_CORES)], axis=0)
    coeffs = np.concatenate([outs[i]["coeffs"] for i in range(N_CORES)], axis=0)
    return rec, coeffs


# revision 10
# speedup vs baseline: 2.1283x; 2.1283x over previous
"""Despawn2D (8-level db-style DWT analysis + synthesis) on 8 Trainium2 cores.

Math: the reference's FFT circular convolutions with 4-tap filters reduce to
4-tap circular stencils (L = 8192 is a power of two, so the ReplicationPad is
a no-op).  Per level:

  analysis:  out[j]  = f0*a[2j] + f1*a[2j-1] + f2*a[2j-2] + f3*a[2j-3] (mod N)
             with f = h (approx) and f = g (detail), g = flip(h)*(+,-,+,-)
  synthesis: out[2i]   = g0*d[i]   + g2*d[i+1] + h0*r[i]   + h2*r[i+1]
             out[2i+1] = g1*d[i+1] + g3*d[i+2] + h1*r[i+1] + h3*r[i+2] (mod m)

Each tap is one fused multiply-accumulate instruction (scalar_tensor_tensor)
over a [128, M] tile; circular wrap is handled with small halo regions.

Sharding: pure data parallel — 2048 rows / 8 cores = 256 rows/core,
processed as 2 tiles of 128 partitions x 8192.

When the provided filter bank is orthogonal (it is for the db2 filters the
reference uses), synthesis(analysis(x)) == x exactly, so the "rec" output is
produced by a DMA copy of the input tile and only the analysis runs on the
compute engines.  A host-side fp64 check of the perfect-reconstruction
property on a small probe vector selects that fast path; otherwise a full
on-device synthesis variant is used.
"""

import numpy as np

LEVELS = 8
L = 8192
ROWS_TOTAL = 2048
N_CORES = 8
RPC = ROWS_TOTAL // N_CORES  # rows per core
P = 128  # SBUF partitions
NT = RPC // P  # tiles per core

_nc_cache = {}


def _make_g(h):
    g = h[::-1].copy()
    g[1::2] *= -1.0
    return g


def _taps_array(scaling):
    """(LEVELS*8,) row: per level [h0..h3, g0..g3], tiled to (P, LEVELS*8)."""
    row = np.empty(LEVELS * 8, np.float32)
    for lev in range(LEVELS):
        h = scaling[lev].astype(np.float32)
        g = _make_g(h)
        row[lev * 8: lev * 8 + 4] = h
        row[lev * 8 + 4: lev * 8 + 8] = g
    return np.tile(row, (P, 1)).copy()


def _pr_is_identity(scaling):
    """fp64 host check: does synthesis(analysis(x)) == x for these filters?"""
    rng = np.random.default_rng(1234)
    n0 = 1 << (LEVELS + 2)
    x = rng.standard_normal((2, n0))
    a = x.copy()
    details = []
    for lev in range(LEVELS):
        h = scaling[lev].astype(np.float64)
        g = _make_g(h)
        N = a.shape[1]
        idx = (np.arange(N // 2)[:, None] * 2 - np.arange(4)[None, :]) % N
        d = (a[:, idx] * g).sum(-1)
        a = (a[:, idx] * h).sum(-1)
        details.append(d)
    r = a
    for lev in reversed(range(LEVELS)):
        h = scaling[lev].astype(np.float64)
        g = _make_g(h)
        d = details[lev]
        m = r.shape[1]
        out = np.empty((2, 2 * m))
        i = np.arange(m)
        out[:, 0::2] = (g[0] * d[:, i] + g[2] * d[:, (i + 1) % m]
                        + h[0] * r[:, i] + h[2] * r[:, (i + 1) % m])
        out[:, 1::2] = (g[1] * d[:, (i + 1) % m] + g[3] * d[:, (i + 2) % m]
                        + h[1] * r[:, (i + 1) % m] + h[3] * r[:, (i + 2) % m])
        r = out
    # scaling arrives as fp32, so an orthogonal filter bank reconstructs to
    # ~1e-8 (fp32 rounding of the filter constants), not fp64 precision.
    # Non-orthogonal filters give O(1) error, so 1e-6 separates cleanly.
    err = np.abs(r - x).max() / max(np.abs(x).max(), 1e-30)
    return err < 1e-6


def _build(synth: bool):
    import concourse.bacc as bacc
    import concourse.mybir as mybir
    from concourse.tile import TileContext

    f32 = mybir.dt.float32
    Alu = mybir.AluOpType

    nc = bacc.Bacc()
    x = nc.dram_tensor("x", [RPC, L], f32, kind="ExternalInput")
    taps = nc.dram_tensor("taps", [P, LEVELS * 8], f32, kind="ExternalInput")
    rec = nc.dram_tensor("rec", [RPC, L], f32, kind="ExternalOutput")
    coeffs = nc.dram_tensor("coeffs", [RPC, L], f32, kind="ExternalOutput")

    # detail block offsets inside a coeffs row: [d0 | d1 | ... | d7 | a8]
    doff = []
    off = 0
    for lev in range(LEVELS):
        doff.append(off)
        off += L >> (lev + 1)
    aoff = off  # 8160

    with TileContext(nc) as tc:
        import contextlib
        with contextlib.ExitStack() as ctx:
            cpool = ctx.enter_context(tc.tile_pool(name="consts", bufs=1))
            xpool = ctx.enter_context(
                tc.tile_pool(name="xio", bufs=1 if synth else 2))
            wpool = ctx.enter_context(tc.tile_pool(name="work", bufs=1))
            spool = ctx.enter_context(tc.tile_pool(name="scratch", bufs=2))

            tp0 = cpool.tile([P, LEVELS * 8], f32)
            nc.sync.dma_start(out=tp0[:, :], in_=taps[:, :])
            # DVE-owned copy: ScalarE (ACT) instructions support only ONE
            # semaphore wait, so everything ACT touches must trace to DVE.
            tp = cpool.tile([P, LEVELS * 8], f32)
            nc.vector.tensor_copy(out=tp[:, :], in_=tp0[:, :])

            def tap(lev, k):  # h taps
                c = lev * 8 + k
                return tp[:, c:c + 1]

            def gtap(lev, k):  # g taps
                c = lev * 8 + 4 + k
                return tp[:, c:c + 1]

            for t in range(NT):
                rows = slice(t * P, (t + 1) * P)

                # input tile with 3-element left halo (circular wrap)
                xt = xpool.tile([P, L + 3], f32, tag="xt")
                nc.sync.dma_start(out=xt[:, 3:3 + L], in_=x[rows, :])
                nc.vector.tensor_copy(out=xt[:, 0:3], in_=xt[:, L:L + 3])

                if not synth:
                    # orthogonal filter bank: synthesis(analysis(x)) == x
                    nc.sync.dma_start(out=rec[rows, :], in_=xt[:, 3:3 + L])

                # ---------------- analysis ----------------
                a_ext = xt  # ext[i] == a[i-3]
                d_tiles = []
                a_last = None
                for lev in range(LEVELS):
                    N = L >> lev
                    M = N >> 1
                    last = lev == LEVELS - 1
                    if not last:
                        a_t = wpool.tile([P, M + 3], f32, tag=f"a{lev}")
                        a_main = a_t[:, 3:3 + M]
                    else:
                        a_t = wpool.tile([P, M + 2], f32, tag=f"a{lev}")
                        a_main = a_t[:, 0:M]
                    d_t = wpool.tile([P, M + 2], f32, tag=f"d{lev}")
                    d_main = d_t[:, 0:M]

                    for out_m, taps4, sl in (
                        (a_main, [tap(lev, k) for k in range(4)], "sa"),
                        (d_main, [gtap(lev, k) for k in range(4)], "sd"),
                    ):
                        # tap k reads ext[(3-k) + 2j], j = 0..M-1
                        # first tap -> scratch on ACT (except level 0, whose
                        # input xt is DMA-written: ACT only allows one wait);
                        # middle taps accumulate scratch on DVE; the last tap
                        # writes the real buffer (WAR vs its DMA reader then
                        # lands on DVE, which allows several waits).
                        s = spool.tile([P, M], f32, tag=sl)
                        if lev == 0:
                            nc.vector.tensor_scalar_mul(
                                s[:, 0:M], a_ext[:, 3:3 + N:2], taps4[0])
                        else:
                            nc.scalar.mul(
                                s[:, 0:M], a_ext[:, 3:3 + N:2], taps4[0])
                        for k in (1, 2):
                            nc.vector.scalar_tensor_tensor(
                                out=s[:, 0:M],
                                in0=a_ext[:, 3 - k:3 - k + N:2],
                                scalar=taps4[k],
                                in1=s[:, 0:M],
                                op0=Alu.mult,
                                op1=Alu.add,
                            )
                        nc.vector.scalar_tensor_tensor(
                            out=out_m,
                            in0=a_ext[:, 0:N:2],
                            scalar=taps4[3],
                            in1=s[:, 0:M],
                            op0=Alu.mult,
                            op1=Alu.add,
                        )

                    # details go straight out to HBM
                    nc.sync.dma_start(
                        out=coeffs[rows, doff[lev]:doff[lev] + M], in_=d_main)
                    if last:
                        nc.sync.dma_start(
                            out=coeffs[rows, aoff:aoff + M], in_=a_main)

                    if not last:
                        # left halo: ext[0:3] = a[M-3:M]
                        nc.vector.tensor_copy(
                            out=a_t[:, 0:3], in_=a_t[:, M:M + 3])
                    elif synth:
                        # right halo for synthesis start
                        nc.vector.tensor_copy(
                            out=a_t[:, M:M + 2], in_=a_t[:, 0:2])
                    if synth and not last:
                        pass  # d halos filled below only when needed
                    d_tiles.append(d_t)
                    a_ext = a_t
                    if last:
                        a_last = a_t

                # ---------------- synthesis ----------------
                if synth:
                    r_ext = a_last  # [P, m+2] with right halo
                    for lev in reversed(range(LEVELS)):
                        m = L >> (lev + 1)
                        d_t = d_tiles[lev]
                        # fill d right halo: d[m:m+2] = d[0:2]
                        nc.vector.tensor_copy(
                            out=d_t[:, m:m + 2], in_=d_t[:, 0:2])
                        if lev > 0:
                            o_t = wpool.tile([P, 2 * m + 2], f32, tag=f"r{lev}")
                            ev = o_t[:, 0:2 * m:2]
                            od = o_t[:, 1:2 * m:2]
                        else:
                            # final level: write into xt, then DMA to rec
                            o_t = xt
                            ev = xt[:, 3:3 + L:2]
                            od = xt[:, 4:4 + L - 1:2]
                        h4 = [tap(lev, k) for k in range(4)]
                        g4 = [gtap(lev, k) for k in range(4)]
                        # stride-2 output APs fault the ACT engine on HW, so
                        # the first tap runs on DVE here (outputs interleave)
                        first_mul = nc.vector.tensor_scalar_mul
                        # even: g0*d[i] + g2*d[i+1] + h0*r[i] + h2*r[i+1]
                        first_mul(ev, d_t[:, 0:m], g4[0])
                        for src, s in ((d_t[:, 1:m + 1], g4[2]),
                                       (r_ext[:, 0:m], h4[0]),
                                       (r_ext[:, 1:m + 1], h4[2])):
                            nc.vector.scalar_tensor_tensor(
                                out=ev, in0=src, scalar=s, in1=ev,
                                op0=Alu.mult, op1=Alu.add)
                        # odd: g1*d[i+1] + g3*d[i+2] + h1*r[i+1] + h3*r[i+2]
                        first_mul(od, d_t[:, 1:m + 1], g4[1])
                        for src, s in ((d_t[:, 2:m + 2], g4[3]),
                                       (r_ext[:, 1:m + 1], h4[1]),
                                       (r_ext[:, 2:m + 2], h4[3])):
                            nc.vector.scalar_tensor_tensor(
                                out=od, in0=src, scalar=s, in1=od,
                                op0=Alu.mult, op1=Alu.add)
                        if lev > 0:
                            nc.vector.tensor_copy(
                                out=o_t[:, 2 * m:2 * m + 2], in_=o_t[:, 0:2])
                            r_ext = o_t
                    nc.sync.dma_start(out=rec[rows, :], in_=xt[:, 3:3 + L])

    nc.finalize()
    return nc


def _get_nc(synth: bool):
    key = ("synth", synth)
    if key not in _nc_cache:
        _nc_cache[key] = _build(synth)
    return _nc_cache[key]


def kernel(x: np.ndarray, scaling: np.ndarray):
    from concourse.bass_utils import run_bass_kernel_spmd

    x = np.ascontiguousarray(np.asarray(x, np.float32))
    scaling = np.asarray(scaling, np.float32)
    assert x.shape == (ROWS_TOTAL, L), x.shape
    assert scaling.shape == (LEVELS, 4), scaling.shape

    synth = not _pr_is_identity(scaling)
    nc = _get_nc(synth)

    taps = _taps_array(scaling)
    in_maps = [
        {"x": np.ascontiguousarray(x[i * RPC:(i + 1) * RPC]), "taps": taps}
        for i in range(N_CORES)
    ]
    res = run_bass_kernel_spmd(nc, in_maps, core_ids=list(range(N_CORES)))
    outs = res.results
    rec = np.concatenate([outs[i]["rec"] for i in range(N_CORES)], axis=0)
    coeffs = np.concatenate([outs[i]["coeffs"] for i in range(N_CORES)], axis=0)
    return rec, coeffs
